# revision 23
# baseline (speedup 1.0000x reference)
"""Fused MergedQKVParallelLinearWithDelta kernel for 8 Trainium2 NeuronCores.

Strategy (tensor-parallel on the QKV output dim, as in vLLM):
  - Each core owns a 768-row output shard (512 q + 128 k + 128 v rows).
  - The host dequantizes the GPTQ 4-bit deltas and MERGES them with the base
    weight: merged[d] = w_base + sc_d * (w4_d - z_d - 1), cast to f16 and laid
    out K-major.  The device then needs a single weight-stationary matmul pass
    per (output-block, adapter): out^T[o, t] = sum_k mergedT[k, o] * xg[k, t].
  - Tokens are sorted by adapter on the host (runs padded to 8) and x is
    pre-gathered/pre-transposed to K-major f16, so the device does no
    gather/transpose/dequant work at all: just DMA + matmul + PSUM evac.
  - Bias is added on the host during unshard.
"""

from contextlib import ExitStack

import numpy as np

import concourse.tile as tile
from concourse import bacc
from concourse import mybir
from concourse.bass_utils import run_bass_kernel_spmd

N_CORES = 8
T, IN = 1024, 4096
Q, KV = 4096, 1024
OUT = Q + 2 * KV
D = 4
OS = OUT // N_CORES          # 768 output rows per core
NOT = OS // 128              # 6 output blocks per core
NB = IN // 128               # 32 K tiles
PAD = 8                      # token-run padding granularity
SEG = 512                    # max PSUM free dim (one f32 bank)
NXCH = 4                     # x is loaded in 4 chunks of 8 K-tiles

F16 = mybir.dt.float16
F32 = mybir.dt.float32


# ---------------------------------------------------------------------------
# Host-side routing schedule
# ---------------------------------------------------------------------------
def _schedule(indices):
    idx = np.asarray(indices).astype(np.int64)
    gather_parts, orig_parts, runs = [], [], []
    t0 = 0
    for d in range(D):
        toks = np.nonzero(idx == d)[0]
        n = len(toks)
        if n == 0:
            continue
        npad = (-n) % PAD
        gather_parts.append(np.concatenate([toks, np.full(npad, toks[0], np.int64)]))
        orig_parts.append(np.concatenate([toks, np.full(npad, -1, np.int64)]))
        ln = n + npad
        runs.append((d, t0, ln))
        t0 += ln
    gather = np.concatenate(gather_parts)
    origs = np.concatenate(orig_parts)
    segs = []
    for d, s0, ln in runs:
        c = 0
        while c < ln:
            segs.append((d, s0 + c, min(SEG, ln - c)))
            c += SEG
    return t0, tuple(segs), gather, origs


# ---------------------------------------------------------------------------
# Device program
# ---------------------------------------------------------------------------
def _build_program(t_pad, segs):
    nc = bacc.Bacc(
        trn_type="TRN2", target_bir_lowering=False, debug=False, num_devices=1
    )
    xg_d = nc.dram_tensor("xg", [128, NB * t_pad], F16, kind="ExternalInput").ap()
    wp_d = nc.dram_tensor("wp", [NOT, 2, 128, 8192], F16, kind="ExternalInput").ap()
    outT_d = nc.dram_tensor("outT", [NOT, 128, t_pad], F16, kind="ExternalOutput").ap()

    kpc = NB // NXCH  # K-tiles per x chunk

    with tile.TileContext(nc) as tc, ExitStack() as ctx:
        px = ctx.enter_context(tc.tile_pool(name="xp", bufs=1))
        pwc = ctx.enter_context(tc.tile_pool(name="wc", bufs=4))
        pwf = ctx.enter_context(tc.tile_pool(name="wf", bufs=4))
        pps = ctx.enter_context(tc.tile_pool(name="ps", bufs=8, space="PSUM"))
        pout = ctx.enter_context(tc.tile_pool(name="op", bufs=6))

        # x chunks (K-major slabs) loaded via the ACT HWDGE ring.  Every psum
        # group can make progress as soon as the first chunk lands, which is
        # what keeps the PE fed while x and weights share HBM bandwidth.
        xch = []
        for j in range(NXCH):
            t = px.tile([128, kpc * t_pad], F16, tag=f"x{j}", name=f"x{j}")
            nc.scalar.dma_start(
                t[:], xg_d[:, kpc * j * t_pad : kpc * (j + 1) * t_pad]
            )
            xch.append(t)

        def emit_mm(ps, wt, fo, kb, s0, ln):
            nc.tensor.matmul(
                ps[:],
                lhsT=wt[:, fo + kb * 128 : fo + (kb + 1) * 128],
                rhs=xch[kb // kpc][
                    :, (kb % kpc) * t_pad + s0 : (kb % kpc) * t_pad + s0 + ln
                ],
                start=(kb == 0),
                stop=(kb == NB - 1),
            )

        # weight DMA granularity: 2-panel halves, except single panels for the
        # last block (shorter compute tail after the DMA stream ends)
        fine = {NOT - 1}
        for ot in range(NOT):
            if ot in fine:
                wts = []
                for dd in range(4):
                    wt = pwf.tile([128, 4096], F16, tag="wtf", name=f"wtf{ot}_{dd}")
                    h, o0 = dd // 2, (dd % 2) * 4096
                    nc.sync.dma_start(wt[:], wp_d[ot, h][:, o0 : o0 + 4096])
                    wts.append(wt)
                getw = lambda d, _w=wts: (_w[d], 0)
            else:
                wts = []
                for h in range(2):
                    wt = pwc.tile([128, 8192], F16, tag="wt", name=f"wt{ot}_{h}")
                    nc.sync.dma_start(wt[:], wp_d[ot, h])
                    wts.append(wt)
                getw = lambda d, _w=wts: (_w[d // 2], (d % 2) * 4096)
            pss = [
                pps.tile([128, ln], F32, tag="ps", space="PSUM", name=f"ps{ot}_{i}")
                for i, (_, _, ln) in enumerate(segs)
            ]
            if ot == 0:
                # kb-outer: consume x chunks in arrival order.  At each chunk
                # boundary, a short burst of tiny dummy matmuls (on already-
                # resident data) keeps the PE non-idle so the HAM clock gate
                # stays at 8/8 through the chunk-arrival gaps.
                dps = pps.tile([128, 64], F32, tag="ps", space="PSUM", name="dps")
                for kb in range(NB):
                    if kb and kb % kpc == 0:
                        prev = xch[kb // kpc - 1]
                        for _ in range(24):
                            nc.tensor.matmul(
                                dps[:],
                                lhsT=wts[0][:, 0:128],
                                rhs=prev[:, 0:64],
                                start=True,
                                stop=True,
                            )
                    for ps, (d, s0, ln) in zip(pss, segs):
                        wt, fo = getw(d)
                        emit_mm(ps, wt, fo, kb, s0, ln)
            else:
                for ps, (d, s0, ln) in zip(pss, segs):
                    wt, fo = getw(d)
                    for kb in range(NB):
                        emit_mm(ps, wt, fo, kb, s0, ln)
            for ps, (d, s0, ln) in zip(pss, segs):
                ob = pout.tile([128, ln], F16, tag="ob")
                nc.vector.tensor_copy(ob[:], ps[:])
                # final block: ACT HWDGE ring (idle by then, lower latency)
                eng = nc.scalar if ot in fine else nc.gpsimd
                eng.dma_start(outT_d[ot, :, s0 : s0 + ln], ob[:])

    nc.compile()
    return nc


# ---------------------------------------------------------------------------
# Host wrapper
# ---------------------------------------------------------------------------
def _unpack_zeros(qz, o_count):
    # qz: [D, o_count//8, 1] int32; returns [D, o_count] float zeros
    o = np.arange(o_count)
    words = qz[:, o >> 3, 0].astype(np.int64)
    return ((words >> (4 * (o & 7))) & 0xF).astype(np.float32)


def _unpack_nibbles(qw):
    # qw: [D, O, K//8] int32 -> [D, O, K] uint8 (nibble k packed at bit 4*(k%8))
    Dd, O, Kp = qw.shape
    b = np.ascontiguousarray(qw).view(np.uint8).reshape(Dd, O, Kp * 4)
    w4 = np.empty((Dd, O, Kp * 8), np.uint8)
    w4[..., 0::2] = b & 0xF
    w4[..., 1::2] = b >> 4
    return w4


_prog_cache = {}


def kernel(**inputs):
    x = np.ascontiguousarray(np.asarray(inputs["x"], dtype=np.float32))
    w_base = np.asarray(inputs["w_base"], dtype=np.float32)
    bias = np.asarray(inputs["bias"], dtype=np.float32)
    qw_q = np.asarray(inputs["qweight_q"], dtype=np.int32)
    qw_k = np.asarray(inputs["qweight_k"], dtype=np.int32)
    qw_v = np.asarray(inputs["qweight_v"], dtype=np.int32)
    qz_q = np.asarray(inputs["qzeros_q"], dtype=np.int32)
    qz_k = np.asarray(inputs["qzeros_k"], dtype=np.int32)
    qz_v = np.asarray(inputs["qzeros_v"], dtype=np.int32)
    sc_q = np.asarray(inputs["scales_q"], dtype=np.float32)
    sc_k = np.asarray(inputs["scales_k"], dtype=np.float32)
    sc_v = np.asarray(inputs["scales_v"], dtype=np.float32)
    indices = np.asarray(inputs["indices"])

    t_pad, segs, gather, origs = _schedule(indices)

    key = (t_pad, segs)
    if key not in _prog_cache:
        _prog_cache[key] = _build_program(t_pad, segs)
    nc = _prog_cache[key]

    # gathered, K-major activations (shared by all cores):
    # xg[p, kb*t_pad + t] = x[gather[t], kb*128+p]
    xs = x[gather].astype(np.float16)                       # [t_pad, IN]
    xg = np.ascontiguousarray(
        xs.T.reshape(NB, 128, t_pad).transpose(1, 0, 2)
    ).reshape(128, NB * t_pad)

    z_q = _unpack_zeros(qz_q, Q)
    z_k = _unpack_zeros(qz_k, KV)
    z_v = _unpack_zeros(qz_v, KV)
    w4_q = _unpack_nibbles(qw_q)
    w4_k = _unpack_nibbles(qw_k)
    w4_v = _unpack_nibbles(qw_v)

    SQ, SK = Q // N_CORES, KV // N_CORES
    in_maps = []
    for c in range(N_CORES):
        qs = slice(SQ * c, SQ * (c + 1))
        ks = slice(SK * c, SK * (c + 1))
        wb = np.concatenate(
            [w_base[qs], w_base[Q + SK * c : Q + SK * (c + 1)],
             w_base[Q + KV + SK * c : Q + KV + SK * (c + 1)]], axis=0
        )                                                    # [OS, IN] f32
        w4 = np.concatenate([w4_q[:, qs], w4_k[:, ks], w4_v[:, ks]], axis=1)
        z = np.concatenate([z_q[:, qs], z_k[:, ks], z_v[:, ks]], axis=1)
        sc = np.concatenate(
            [sc_q[:, qs, 0], sc_k[:, ks, 0], sc_v[:, ks, 0]], axis=1
        )                                                    # [D, OS]
        # merged[d] = wb + sc_d * w4_d - sc_d*(z_d+1)
        merged = np.empty((D, OS, IN), np.float16)
        for d in range(D):
            md = w4[d].astype(np.float32) * sc[d][:, None]
            md += wb
            md -= (sc[d] * (z[d] + 1.0))[:, None]
            merged[d] = md.astype(np.float16)
        # wp[ot, h, kk, dd*4096 + kb*128 + oo] = merged[2h+dd, ot*128+oo, kb*128+kk]
        tmp = merged.reshape(2, 2, NOT, 128, NB, 128)        # (h, dd, ot, oo, kb, kk)
        wp = np.ascontiguousarray(tmp.transpose(2, 0, 5, 1, 4, 3)).reshape(
            NOT, 2, 128, 8192
        )
        in_maps.append({"xg": xg, "wp": wp})

    import os

    trace = bool(int(os.environ.get("KERNEL_TRACE", "0")))
    res = run_bass_kernel_spmd(
        nc, in_maps, core_ids=list(range(N_CORES)), trace=trace
    )
    kernel._last_results = res

    out = np.empty([T, OUT], np.float32)
    valid = origs >= 0
    vpos = np.nonzero(valid)[0]
    vtok = origs[valid]
    for c in range(N_CORES):
        r = np.asarray(res.results[c]["outT"]).reshape(OS, t_pad)
        cols = np.concatenate(
            [
                np.arange(SQ * c, SQ * (c + 1)),
                np.arange(Q + SK * c, Q + SK * (c + 1)),
                np.arange(Q + KV + SK * c, Q + KV + SK * (c + 1)),
            ]
        )
        out[vtok[:, None], cols[None, :]] = r.T[vpos].astype(np.float32)
    out += bias[None, :]
    return out


# revision 25
# speedup vs baseline: 1.0197x; 1.0197x over previous
"""Fused MergedQKVParallelLinearWithDelta kernel for 8 Trainium2 NeuronCores.

Strategy (tensor-parallel on the QKV output dim, as in vLLM):
  - Each core owns a 768-row output shard (512 q + 128 k + 128 v rows).
  - The host dequantizes the GPTQ 4-bit deltas and MERGES them with the base
    weight: merged[d] = w_base + sc_d * (w4_d - z_d - 1), cast to f16 and laid
    out K-major.  The device then needs a single weight-stationary matmul pass
    per (output-block, adapter): out^T[o, t] = sum_k mergedT[k, o] * xg[k, t].
  - Tokens are sorted by adapter on the host (runs padded to 8) and x is
    pre-gathered/pre-transposed to K-major f16, so the device does no
    gather/transpose/dequant work at all: just DMA + matmul + PSUM evac.
  - Bias is added on the host during unshard.
"""

from contextlib import ExitStack

import numpy as np

import concourse.tile as tile
from concourse import bacc
from concourse import mybir
from concourse.bass_utils import run_bass_kernel_spmd

N_CORES = 8
T, IN = 1024, 4096
Q, KV = 4096, 1024
OUT = Q + 2 * KV
D = 4
OS = OUT // N_CORES          # 768 output rows per core
NOT = OS // 128              # 6 output blocks per core
NB = IN // 128               # 32 K tiles
PAD = 4                      # token-run padding granularity
SEG = 512                    # max PSUM free dim (one f32 bank)
NXCH = 4                     # x is loaded in 4 chunks of 8 K-tiles

F16 = mybir.dt.float16
F32 = mybir.dt.float32


# ---------------------------------------------------------------------------
# Host-side routing schedule
# ---------------------------------------------------------------------------
def _schedule(indices):
    idx = np.asarray(indices).astype(np.int64)
    gather_parts, orig_parts, runs = [], [], []
    t0 = 0
    for d in range(D):
        toks = np.nonzero(idx == d)[0]
        n = len(toks)
        if n == 0:
            continue
        npad = (-n) % PAD
        gather_parts.append(np.concatenate([toks, np.full(npad, toks[0], np.int64)]))
        orig_parts.append(np.concatenate([toks, np.full(npad, -1, np.int64)]))
        ln = n + npad
        runs.append((d, t0, ln))
        t0 += ln
    gather = np.concatenate(gather_parts)
    origs = np.concatenate(orig_parts)
    segs = []
    for d, s0, ln in runs:
        c = 0
        while c < ln:
            segs.append((d, s0 + c, min(SEG, ln - c)))
            c += SEG
    return t0, tuple(segs), gather, origs


# ---------------------------------------------------------------------------
# Device program
# ---------------------------------------------------------------------------
def _build_program(t_pad, segs):
    nc = bacc.Bacc(
        trn_type="TRN2", target_bir_lowering=False, debug=False, num_devices=1
    )
    xg_d = nc.dram_tensor("xg", [128, NB * t_pad], F16, kind="ExternalInput").ap()
    wp_d = nc.dram_tensor("wp", [NOT, 2, 128, 8192], F16, kind="ExternalInput").ap()
    outT_d = nc.dram_tensor("outT", [NOT, 128, t_pad], F16, kind="ExternalOutput").ap()

    kpc = NB // NXCH  # K-tiles per x chunk

    with tile.TileContext(nc) as tc, ExitStack() as ctx:
        px = ctx.enter_context(tc.tile_pool(name="xp", bufs=1))
        pwc = ctx.enter_context(tc.tile_pool(name="wc", bufs=5))
        pwf = ctx.enter_context(tc.tile_pool(name="wf", bufs=4))
        pps = ctx.enter_context(tc.tile_pool(name="ps", bufs=8, space="PSUM"))
        pout = ctx.enter_context(tc.tile_pool(name="op", bufs=6))

        # x chunks (K-major slabs) loaded via the ACT HWDGE ring.  Every psum
        # group can make progress as soon as the first chunk lands, which is
        # what keeps the PE fed while x and weights share HBM bandwidth.
        xch = []
        for j in range(NXCH):
            t = px.tile([128, kpc * t_pad], F16, tag=f"x{j}", name=f"x{j}")
            nc.scalar.dma_start(
                t[:], xg_d[:, kpc * j * t_pad : kpc * (j + 1) * t_pad]
            )
            xch.append(t)

        def emit_mm(ps, wt, fo, kb, s0, ln):
            nc.tensor.matmul(
                ps[:],
                lhsT=wt[:, fo + kb * 128 : fo + (kb + 1) * 128],
                rhs=xch[kb // kpc][
                    :, (kb % kpc) * t_pad + s0 : (kb % kpc) * t_pad + s0 + ln
                ],
                start=(kb == 0),
                stop=(kb == NB - 1),
            )

        # weight DMA granularity: 2-panel halves, except single panels for the
        # last block (shorter compute tail after the DMA stream ends)
        fine = {NOT - 1}
        for ot in range(NOT):
            if ot in fine:
                wts = []
                for dd in range(4):
                    wt = pwf.tile([128, 4096], F16, tag="wtf", name=f"wtf{ot}_{dd}")
                    h, o0 = dd // 2, (dd % 2) * 4096
                    nc.sync.dma_start(wt[:], wp_d[ot, h][:, o0 : o0 + 4096])
                    wts.append(wt)
                getw = lambda d, _w=wts: (_w[d], 0)
            else:
                wts = []
                for h in range(2):
                    wt = pwc.tile([128, 8192], F16, tag="wt", name=f"wt{ot}_{h}")
                    nc.sync.dma_start(wt[:], wp_d[ot, h])
                    wts.append(wt)
                getw = lambda d, _w=wts: (_w[d // 2], (d % 2) * 4096)
            pss = [
                pps.tile([128, ln], F32, tag="ps", space="PSUM", name=f"ps{ot}_{i}")
                for i, (_, _, ln) in enumerate(segs)
            ]
            if ot == 0:
                # kb-outer: consume x chunks in arrival order
                for kb in range(NB):
                    for ps, (d, s0, ln) in zip(pss, segs):
                        wt, fo = getw(d)
                        emit_mm(ps, wt, fo, kb, s0, ln)
            else:
                for ps, (d, s0, ln) in zip(pss, segs):
                    wt, fo = getw(d)
                    for kb in range(NB):
                        emit_mm(ps, wt, fo, kb, s0, ln)
            for ps, (d, s0, ln) in zip(pss, segs):
                ob = pout.tile([128, ln], F16, tag="ob")
                nc.vector.tensor_copy(ob[:], ps[:])
                # ACT HWDGE ring: queued behind the x loads, cheaper per-store
                # latency than SWDGE, and leaves GpSimd entirely unused
                nc.scalar.dma_start(outT_d[ot, :, s0 : s0 + ln], ob[:])

    nc.compile()
    return nc


# ---------------------------------------------------------------------------
# Host wrapper
# ---------------------------------------------------------------------------
def _unpack_zeros(qz, o_count):
    # qz: [D, o_count//8, 1] int32; returns [D, o_count] float zeros
    o = np.arange(o_count)
    words = qz[:, o >> 3, 0].astype(np.int64)
    return ((words >> (4 * (o & 7))) & 0xF).astype(np.float32)


def _unpack_nibbles(qw):
    # qw: [D, O, K//8] int32 -> [D, O, K] uint8 (nibble k packed at bit 4*(k%8))
    Dd, O, Kp = qw.shape
    b = np.ascontiguousarray(qw).view(np.uint8).reshape(Dd, O, Kp * 4)
    w4 = np.empty((Dd, O, Kp * 8), np.uint8)
    w4[..., 0::2] = b & 0xF
    w4[..., 1::2] = b >> 4
    return w4


_prog_cache = {}


def kernel(**inputs):
    x = np.ascontiguousarray(np.asarray(inputs["x"], dtype=np.float32))
    w_base = np.asarray(inputs["w_base"], dtype=np.float32)
    bias = np.asarray(inputs["bias"], dtype=np.float32)
    qw_q = np.asarray(inputs["qweight_q"], dtype=np.int32)
    qw_k = np.asarray(inputs["qweight_k"], dtype=np.int32)
    qw_v = np.asarray(inputs["qweight_v"], dtype=np.int32)
    qz_q = np.asarray(inputs["qzeros_q"], dtype=np.int32)
    qz_k = np.asarray(inputs["qzeros_k"], dtype=np.int32)
    qz_v = np.asarray(inputs["qzeros_v"], dtype=np.int32)
    sc_q = np.asarray(inputs["scales_q"], dtype=np.float32)
    sc_k = np.asarray(inputs["scales_k"], dtype=np.float32)
    sc_v = np.asarray(inputs["scales_v"], dtype=np.float32)
    indices = np.asarray(inputs["indices"])

    t_pad, segs, gather, origs = _schedule(indices)

    key = (t_pad, segs)
    if key not in _prog_cache:
        _prog_cache[key] = _build_program(t_pad, segs)
    nc = _prog_cache[key]

    # gathered, K-major activations (shared by all cores):
    # xg[p, kb*t_pad + t] = x[gather[t], kb*128+p]
    xs = x[gather].astype(np.float16)                       # [t_pad, IN]
    xg = np.ascontiguousarray(
        xs.T.reshape(NB, 128, t_pad).transpose(1, 0, 2)
    ).reshape(128, NB * t_pad)

    z_q = _unpack_zeros(qz_q, Q)
    z_k = _unpack_zeros(qz_k, KV)
    z_v = _unpack_zeros(qz_v, KV)
    w4_q = _unpack_nibbles(qw_q)
    w4_k = _unpack_nibbles(qw_k)
    w4_v = _unpack_nibbles(qw_v)

    SQ, SK = Q // N_CORES, KV // N_CORES
    in_maps = []
    for c in range(N_CORES):
        qs = slice(SQ * c, SQ * (c + 1))
        ks = slice(SK * c, SK * (c + 1))
        wb = np.concatenate(
            [w_base[qs], w_base[Q + SK * c : Q + SK * (c + 1)],
             w_base[Q + KV + SK * c : Q + KV + SK * (c + 1)]], axis=0
        )                                                    # [OS, IN] f32
        w4 = np.concatenate([w4_q[:, qs], w4_k[:, ks], w4_v[:, ks]], axis=1)
        z = np.concatenate([z_q[:, qs], z_k[:, ks], z_v[:, ks]], axis=1)
        sc = np.concatenate(
            [sc_q[:, qs, 0], sc_k[:, ks, 0], sc_v[:, ks, 0]], axis=1
        )                                                    # [D, OS]
        # merged[d] = wb + sc_d * w4_d - sc_d*(z_d+1)
        merged = np.empty((D, OS, IN), np.float16)
        for d in range(D):
            md = w4[d].astype(np.float32) * sc[d][:, None]
            md += wb
            md -= (sc[d] * (z[d] + 1.0))[:, None]
            merged[d] = md.astype(np.float16)
        # wp[ot, h, kk, dd*4096 + kb*128 + oo] = merged[2h+dd, ot*128+oo, kb*128+kk]
        tmp = merged.reshape(2, 2, NOT, 128, NB, 128)        # (h, dd, ot, oo, kb, kk)
        wp = np.ascontiguousarray(tmp.transpose(2, 0, 5, 1, 4, 3)).reshape(
            NOT, 2, 128, 8192
        )
        in_maps.append({"xg": xg, "wp": wp})

    import os

    trace = bool(int(os.environ.get("KERNEL_TRACE", "0")))
    res = run_bass_kernel_spmd(
        nc, in_maps, core_ids=list(range(N_CORES)), trace=trace
    )
    kernel._last_results = res

    out = np.empty([T, OUT], np.float32)
    valid = origs >= 0
    vpos = np.nonzero(valid)[0]
    vtok = origs[valid]
    for c in range(N_CORES):
        r = np.asarray(res.results[c]["outT"]).reshape(OS, t_pad)
        cols = np.concatenate(
            [
                np.arange(SQ * c, SQ * (c + 1)),
                np.arange(Q + SK * c, Q + SK * (c + 1)),
                np.arange(Q + KV + SK * c, Q + KV + SK * (c + 1)),
            ]
        )
        out[vtok[:, None], cols[None, :]] = r.T[vpos].astype(np.float32)
    out += bias[None, :]
    return out


# revision 26
# speedup vs baseline: 1.0919x; 1.0708x over previous
"""Fused MergedQKVParallelLinearWithDelta kernel for 8 Trainium2 NeuronCores.

Strategy (tensor-parallel on the QKV output dim, as in vLLM):
  - Each core owns a 768-row output shard (512 q + 128 k + 128 v rows).
  - The host dequantizes the GPTQ 4-bit deltas and MERGES them with the base
    weight: merged[d] = w_base + sc_d * (w4_d - z_d - 1), cast to f16 and laid
    out K-major.  The device then needs a single weight-stationary matmul pass
    per (output-block, adapter): out^T[o, t] = sum_k mergedT[k, o] * xg[k, t].
  - Tokens are sorted by adapter on the host (runs padded to 8) and x is
    pre-gathered/pre-transposed to K-major f16, so the device does no
    gather/transpose/dequant work at all: just DMA + matmul + PSUM evac.
  - Bias is added on the host during unshard.
"""

from contextlib import ExitStack

import numpy as np

import concourse.tile as tile
from concourse import bacc
from concourse import mybir
from concourse.bass_utils import run_bass_kernel_spmd

N_CORES = 8
T, IN = 1024, 4096
Q, KV = 4096, 1024
OUT = Q + 2 * KV
D = 4
OS = OUT // N_CORES          # 768 output rows per core
NOT = OS // 128              # 6 output blocks per core
NB = IN // 128               # 32 K tiles
PAD = 8                      # token-run padding granularity
SEG = 512                    # max PSUM free dim (one f32 bank)
NXCH = 4                     # x is loaded in 4 chunks of 8 K-tiles

F16 = mybir.dt.float16
F32 = mybir.dt.float32


# ---------------------------------------------------------------------------
# Host-side routing schedule
# ---------------------------------------------------------------------------
def _schedule(indices):
    idx = np.asarray(indices).astype(np.int64)
    gather_parts, orig_parts, runs = [], [], []
    t0 = 0
    for d in range(D):
        toks = np.nonzero(idx == d)[0]
        n = len(toks)
        if n == 0:
            continue
        npad = (-n) % PAD
        gather_parts.append(np.concatenate([toks, np.full(npad, toks[0], np.int64)]))
        orig_parts.append(np.concatenate([toks, np.full(npad, -1, np.int64)]))
        ln = n + npad
        runs.append((d, t0, ln))
        t0 += ln
    gather = np.concatenate(gather_parts)
    origs = np.concatenate(orig_parts)
    segs = []
    for d, s0, ln in runs:
        c = 0
        while c < ln:
            segs.append((d, s0 + c, min(SEG, ln - c)))
            c += SEG
    return t0, tuple(segs), gather, origs


# ---------------------------------------------------------------------------
# Device program
# ---------------------------------------------------------------------------
def _build_program(t_pad, segs):
    nc = bacc.Bacc(
        trn_type="TRN2", target_bir_lowering=False, debug=False, num_devices=1
    )
    xg_d = nc.dram_tensor("xg", [128, NB * t_pad], F16, kind="ExternalInput").ap()
    wp_d = nc.dram_tensor("wp", [NOT, 2, 128, 8192], F16, kind="ExternalInput").ap()
    outT_d = nc.dram_tensor("outT", [NOT, 128, t_pad], F16, kind="ExternalOutput").ap()

    kpc = NB // NXCH  # K-tiles per x chunk

    with tile.TileContext(nc) as tc, ExitStack() as ctx:
        px = ctx.enter_context(tc.tile_pool(name="xp", bufs=1))
        pwc = ctx.enter_context(tc.tile_pool(name="wc", bufs=4))
        pwf = ctx.enter_context(tc.tile_pool(name="wf", bufs=4))
        pps = ctx.enter_context(tc.tile_pool(name="ps", bufs=8, space="PSUM"))
        pout = ctx.enter_context(tc.tile_pool(name="op", bufs=6))

        # x chunks (K-major slabs) loaded via the ACT HWDGE ring.  Every psum
        # group can make progress as soon as the first chunk lands, which is
        # what keeps the PE fed while x and weights share HBM bandwidth.
        xch = []
        for j in range(NXCH):
            t = px.tile([128, kpc * t_pad], F16, tag=f"x{j}", name=f"x{j}")
            nc.scalar.dma_start(
                t[:], xg_d[:, kpc * j * t_pad : kpc * (j + 1) * t_pad]
            )
            xch.append(t)

        def emit_mm(ps, wt, fo, kb, s0, ln):
            nc.tensor.matmul(
                ps[:],
                lhsT=wt[:, fo + kb * 128 : fo + (kb + 1) * 128],
                rhs=xch[kb // kpc][
                    :, (kb % kpc) * t_pad + s0 : (kb % kpc) * t_pad + s0 + ln
                ],
                start=(kb == 0),
                stop=(kb == NB - 1),
            )

        # weight DMA granularity: 2-panel halves, except single panels for the
        # last block (shorter compute tail after the DMA stream ends)
        fine = {NOT - 1}
        for ot in range(NOT):
            if ot in fine:
                wts = []
                for dd in range(4):
                    wt = pwf.tile([128, 4096], F16, tag="wtf", name=f"wtf{ot}_{dd}")
                    h, o0 = dd // 2, (dd % 2) * 4096
                    nc.sync.dma_start(wt[:], wp_d[ot, h][:, o0 : o0 + 4096])
                    wts.append(wt)
                getw = lambda d, _w=wts: (_w[d], 0)
            else:
                wts = []
                for h in range(2):
                    wt = pwc.tile([128, 8192], F16, tag="wt", name=f"wt{ot}_{h}")
                    nc.sync.dma_start(wt[:], wp_d[ot, h])
                    wts.append(wt)
                getw = lambda d, _w=wts: (_w[d // 2], (d % 2) * 4096)
            pss = [
                pps.tile([128, ln], F32, tag="ps", space="PSUM", name=f"ps{ot}_{i}")
                for i, (_, _, ln) in enumerate(segs)
            ]
            if ot == 0:
                # kb-outer: consume x chunks in arrival order
                for kb in range(NB):
                    for ps, (d, s0, ln) in zip(pss, segs):
                        wt, fo = getw(d)
                        emit_mm(ps, wt, fo, kb, s0, ln)
            else:
                for ps, (d, s0, ln) in zip(pss, segs):
                    wt, fo = getw(d)
                    for kb in range(NB):
                        emit_mm(ps, wt, fo, kb, s0, ln)
            for ps, (d, s0, ln) in zip(pss, segs):
                ob = pout.tile([128, ln], F16, tag="ob")
                nc.vector.tensor_copy(ob[:], ps[:])
                # final block: ACT HWDGE ring (idle by then, lower latency)
                eng = nc.scalar if ot in fine else nc.gpsimd
                eng.dma_start(outT_d[ot, :, s0 : s0 + ln], ob[:])

    nc.compile()
    return nc


# ---------------------------------------------------------------------------
# Host wrapper
# ---------------------------------------------------------------------------
def _unpack_zeros(qz, o_count):
    # qz: [D, o_count//8, 1] int32; returns [D, o_count] float zeros
    o = np.arange(o_count)
    words = qz[:, o >> 3, 0].astype(np.int64)
    return ((words >> (4 * (o & 7))) & 0xF).astype(np.float32)


def _unpack_nibbles(qw):
    # qw: [D, O, K//8] int32 -> [D, O, K] uint8 (nibble k packed at bit 4*(k%8))
    Dd, O, Kp = qw.shape
    b = np.ascontiguousarray(qw).view(np.uint8).reshape(Dd, O, Kp * 4)
    w4 = np.empty((Dd, O, Kp * 8), np.uint8)
    w4[..., 0::2] = b & 0xF
    w4[..., 1::2] = b >> 4
    return w4


_prog_cache = {}


def kernel(**inputs):
    x = np.ascontiguousarray(np.asarray(inputs["x"], dtype=np.float32))
    w_base = np.asarray(inputs["w_base"], dtype=np.float32)
    bias = np.asarray(inputs["bias"], dtype=np.float32)
    qw_q = np.asarray(inputs["qweight_q"], dtype=np.int32)
    qw_k = np.asarray(inputs["qweight_k"], dtype=np.int32)
    qw_v = np.asarray(inputs["qweight_v"], dtype=np.int32)
    qz_q = np.asarray(inputs["qzeros_q"], dtype=np.int32)
    qz_k = np.asarray(inputs["qzeros_k"], dtype=np.int32)
    qz_v = np.asarray(inputs["qzeros_v"], dtype=np.int32)
    sc_q = np.asarray(inputs["scales_q"], dtype=np.float32)
    sc_k = np.asarray(inputs["scales_k"], dtype=np.float32)
    sc_v = np.asarray(inputs["scales_v"], dtype=np.float32)
    indices = np.asarray(inputs["indices"])

    t_pad, segs, gather, origs = _schedule(indices)

    key = (t_pad, segs)
    if key not in _prog_cache:
        _prog_cache[key] = _build_program(t_pad, segs)
    nc = _prog_cache[key]

    # gathered, K-major activations (shared by all cores):
    # xg[p, kb*t_pad + t] = x[gather[t], kb*128+p]
    xs = x[gather].astype(np.float16)                       # [t_pad, IN]
    xg = np.ascontiguousarray(
        xs.T.reshape(NB, 128, t_pad).transpose(1, 0, 2)
    ).reshape(128, NB * t_pad)

    z_q = _unpack_zeros(qz_q, Q)
    z_k = _unpack_zeros(qz_k, KV)
    z_v = _unpack_zeros(qz_v, KV)
    w4_q = _unpack_nibbles(qw_q)
    w4_k = _unpack_nibbles(qw_k)
    w4_v = _unpack_nibbles(qw_v)

    SQ, SK = Q // N_CORES, KV // N_CORES
    in_maps = []
    for c in range(N_CORES):
        qs = slice(SQ * c, SQ * (c + 1))
        ks = slice(SK * c, SK * (c + 1))
        wb = np.concatenate(
            [w_base[qs], w_base[Q + SK * c : Q + SK * (c + 1)],
             w_base[Q + KV + SK * c : Q + KV + SK * (c + 1)]], axis=0
        )                                                    # [OS, IN] f32
        w4 = np.concatenate([w4_q[:, qs], w4_k[:, ks], w4_v[:, ks]], axis=1)
        z = np.concatenate([z_q[:, qs], z_k[:, ks], z_v[:, ks]], axis=1)
        sc = np.concatenate(
            [sc_q[:, qs, 0], sc_k[:, ks, 0], sc_v[:, ks, 0]], axis=1
        )                                                    # [D, OS]
        # merged[d] = wb + sc_d * w4_d - sc_d*(z_d+1)
        merged = np.empty((D, OS, IN), np.float16)
        for d in range(D):
            md = w4[d].astype(np.float32) * sc[d][:, None]
            md += wb
            md -= (sc[d] * (z[d] + 1.0))[:, None]
            merged[d] = md.astype(np.float16)
        # wp[ot, h, kk, dd*4096 + kb*128 + oo] = merged[2h+dd, ot*128+oo, kb*128+kk]
        tmp = merged.reshape(2, 2, NOT, 128, NB, 128)        # (h, dd, ot, oo, kb, kk)
        wp = np.ascontiguousarray(tmp.transpose(2, 0, 5, 1, 4, 3)).reshape(
            NOT, 2, 128, 8192
        )
        in_maps.append({"xg": xg, "wp": wp})

    import os

    trace = bool(int(os.environ.get("KERNEL_TRACE", "0")))
    res = run_bass_kernel_spmd(
        nc, in_maps, core_ids=list(range(N_CORES)), trace=trace
    )
    kernel._last_results = res

    out = np.empty([T, OUT], np.float32)
    valid = origs >= 0
    vpos = np.nonzero(valid)[0]
    vtok = origs[valid]
    for c in range(N_CORES):
        r = np.asarray(res.results[c]["outT"]).reshape(OS, t_pad)
        cols = np.concatenate(
            [
                np.arange(SQ * c, SQ * (c + 1)),
                np.arange(Q + SK * c, Q + SK * (c + 1)),
                np.arange(Q + KV + SK * c, Q + KV + SK * (c + 1)),
            ]
        )
        out[vtok[:, None], cols[None, :]] = r.T[vpos].astype(np.float32)
    out += bias[None, :]
    return out
